# revision 9
# baseline (speedup 1.0000x reference)
"""CapsuleLayer dynamic-routing kernel for 8 Trainium2 NeuronCores.

Problem: x[32, 2048, 16], W[1, 2048, 64, 32, 16] -> v[32, 64, 32]
  u_hat = einsum('iodk,bik->biod', W[0], x)
  3 routing iterations (softmax over out_caps, squash over out_dim).

Sharding: in_caps (i) split 8 ways (256/core).  W shard is SBUF-resident in
bf16 with (d, o) column order so every DVE broadcast has innermost stride 1
(2x_1p mode).  Per routing pass the tensor engine recomputes u_hat with
consolidated K=64/M=128 matmuls (block-diagonal x in lhsT packs 4 in_caps per
matmul); the scalar engine evacuates PSUM to bf16 and folds the softmax
normalization (cB = exp(agr) * rZ); the vector engine runs all-bf16
tensor_tensor ops at 2x plus a halving-tree d-reduction (first halving on
gpsimd).  The quad loop is software-pipelined: stage A (u_hat matmuls, evac,
x V multiply) runs one quad ahead of stage B (reduce, softmax, c-weight,
selector matmul) so no engine queue blocks in-order on a cross-engine dep.
s_j is AllReduced across the 8 cores in bf16 (pass 1 splits the reduction in
half to overlap the first AR with the second half of the matmuls); squash +
softmax bookkeeping is replicated on every core.

Routing state trick: b_ij(t) = sum_d u_hat * (v_0+...+v_{t-1}), so no b_ij
state is carried - only the accumulated V (32x2048 bf16).
"""

import numpy as np
import ml_dtypes

B, IC, KD, OC, OD = 32, 2048, 16, 64, 32     # batch, in_caps, in_dim, out_caps, out_dim
NCORES = 8
ICC = IC // NCORES                            # 256 in_caps per core
NJ = ICC // 8                                 # 32 j-blocks (8 i per block)
OD2 = OC * OD                                 # 2048 flattened (d, o) columns
NUM_ROUTES = 3
NQ = 2 * NJ                                   # 64 quads (4 i each) per pass

_CACHE = {}


def _build_program():
    import concourse.bacc as bacc
    import concourse.tile as tile
    import concourse.mybir as mybir

    f32 = mybir.dt.float32
    bf16 = mybir.dt.bfloat16
    ALU = mybir.AluOpType
    ACTF = mybir.ActivationFunctionType

    nc = bacc.Bacc("TRN2", target_bir_lowering=False, debug=False, num_devices=NCORES)

    WL_d = nc.dram_tensor("WL", [128, NJ * OD2], bf16, kind="ExternalInput").ap()
    XG_d = nc.dram_tensor("XG", [128, NJ * 128], bf16, kind="ExternalInput").ap()
    X2_d = nc.dram_tensor("X2", [128, NJ * B], bf16, kind="ExternalInput").ap()
    SEL1_d = nc.dram_tensor("SEL1", [128, 32], bf16, kind="ExternalInput").ap()
    vout_d = nc.dram_tensor("v_out", [B, OD2], f32, kind="ExternalOutput").ap()

    with tile.TileContext(nc) as tc:
        with (
            tc.tile_pool(name="const", bufs=1) as cp,
            tc.tile_pool(name="uh", bufs=4) as up,
            tc.tile_pool(name="work", bufs=2) as wp,
            tc.tile_pool(name="small", bufs=2) as sp,
            tc.tile_pool(name="psum", bufs=1, space="PSUM") as pp,
            tc.tile_pool(name="dram", bufs=1, space="DRAM") as dp,
        ):
            # ---- resident inputs ----
            wl = cp.tile([128, NJ * OD2], bf16, tag="wl")
            for blk in range(8):
                w = NJ * OD2 // 8
                nc.sync.dma_start(out=wl[:, blk * w:(blk + 1) * w],
                                  in_=WL_d[:, blk * w:(blk + 1) * w])
            xg = cp.tile([128, NJ * 128], bf16, tag="xg")
            nc.sync.dma_start(out=xg[:, :], in_=XG_d[:, :])
            x2t = cp.tile([128, NJ * B], bf16, tag="x2t")
            nc.sync.dma_start(out=x2t[:, :], in_=X2_d[:, :])
            sel1 = cp.tile([128, 32], bf16, tag="sel1")
            nc.sync.dma_start(out=sel1[:, :], in_=SEL1_d[:, :])

            # ---- persistent state ----
            V4 = cp.tile([128, OD2], bf16, tag="V4")     # V replicated x4 part-groups
            Vaccb = cp.tile([B, OD2], bf16, tag="Vacc")  # running sum of v_t

            NAR = NUM_ROUTES + 1                          # pass-1 uses 2 half ARs
            ar_in = [dp.tile([B, OD2], bf16, tag=f"ari{t}", name=f"ari{t}") for t in range(NAR)]
            ar_out = [dp.tile([B, OD2], bf16, tag=f"aro{t}", name=f"aro{t}") for t in range(NAR)]

            def start_ar(t, src_psum, dst_sb):
                """Evacuate s-partial (psum, f32) -> bf16 sbuf -> allreduce."""
                nc.scalar.copy(dst_sb[:, :], src_psum[0:B, :])
                nc.sync.dma_start(out=ar_in[t][:, :], in_=dst_sb[:, :])
                nc.gpsimd.collective_compute(
                    "AllReduce", ALU.add,
                    replica_groups=[list(range(NCORES))],
                    ins=[ar_in[t].opt()],
                    outs=[ar_out[t].opt()],
                )

            def squash(t, s_sb):
                """v_t = squash(s_sb).  t<2: Vaccb += v_t, V4 <- replicate(Vaccb).
                t==2: DMA v_t to output in (o, d) order."""
                n2 = sp.tile([B, OC], f32, tag="n2", bufs=1)
                if t < NUM_ROUTES - 1:
                    # bf16 halving tree for ||s||^2 (fast; feeds agreements only)
                    sq = wp.tile([B, OD2], bf16, tag="tmp", name=f"sq{t}")
                    nc.vector.tensor_mul(sq[:, :], s_sb[:, :], s_sb[:, :])
                    sh1 = sp.tile([B, OD2 // 2], bf16, tag="sh1", bufs=1)
                    nc.vector.tensor_add(sh1[:, :], sq[:, :OD2 // 2], sq[:, OD2 // 2:])
                    sh2 = sp.tile([B, OD2 // 4], bf16, tag="sh2", bufs=1)
                    nc.vector.tensor_add(sh2[:, :], sh1[:, :OD2 // 4], sh1[:, OD2 // 4:])
                    nc.vector.tensor_reduce(
                        n2[:, :], sh2[:, :].rearrange("p (d o) -> p o d", o=OC),
                        axis=mybir.AxisListType.X, op=ALU.add)
                else:
                    # exact f32 path for the final output
                    sqf = wp.tile([B, OD2], f32, tag="fin32", name=f"sqf{t}", bufs=1)
                    nc.scalar.activation(sqf[:, :], s_sb[:, :], ACTF.Square)
                    nc.vector.tensor_reduce(
                        n2[:, :], sqf[:, :].rearrange("p (d o) -> p o d", o=OC),
                        axis=mybir.AxisListType.X, op=ALU.add)
                r0 = sp.tile([B, OC], f32, tag="r0", bufs=1)
                nc.scalar.activation(r0[:, :], n2[:, :], ACTF.Sqrt)
                # Newton polish: n = 0.5 * (r0 + n2 / r0)
                t1 = sp.tile([B, OC], f32, tag="t1", bufs=1)
                nc.vector.reciprocal(t1[:, :], r0[:, :])
                nc.vector.tensor_mul(t1[:, :], t1[:, :], n2[:, :])
                t2 = sp.tile([B, OC], f32, tag="t2", bufs=1)
                nc.vector.tensor_add(t2[:, :], t1[:, :], r0[:, :])
                nn = sp.tile([B, OC], f32, tag="nn", bufs=1)
                nc.vector.tensor_scalar_mul(nn[:, :], t2[:, :], 0.5)   # |s|
                den = sp.tile([B, OC], f32, tag="den", bufs=1)
                nc.vector.tensor_scalar_add(den[:, :], n2[:, :], 1.0)
                rec = sp.tile([B, OC], f32, tag="rec", bufs=1)
                nc.vector.reciprocal(rec[:, :], den[:, :])
                qq = sp.tile([B, OC], bf16, tag="qq", bufs=1)
                nc.vector.tensor_mul(qq[:, :], nn[:, :], rec[:, :])  # |s|/(1+|s|^2)
                if t == NUM_ROUTES - 1:
                    vtf = wp.tile([B, OD2], f32, tag="fin32", name="vtf", bufs=1)
                    nc.vector.tensor_tensor(
                        out=vtf[:, :].rearrange("p (o d) -> p o d", d=OD),
                        in0=s_sb[:, :].rearrange("p (d o) -> p o d", o=OC),
                        in1=qq[:, :].unsqueeze(2).broadcast_to([B, OC, OD]),
                        op=ALU.mult)
                    nc.sync.dma_start(out=vout_d[:, :], in_=vtf[:, :])
                else:
                    if t == 0:
                        nc.vector.tensor_tensor(
                            out=Vaccb[:, :].rearrange("p (d o) -> p d o", o=OC),
                            in0=s_sb[:, :].rearrange("p (d o) -> p d o", o=OC),
                            in1=qq[:, :].unsqueeze(1).broadcast_to([B, OD, OC]),
                            op=ALU.mult)
                    else:
                        vt = wp.tile([B, OD2], bf16, tag="tmp", name=f"vt{t}")
                        nc.vector.tensor_tensor(
                            out=vt[:, :].rearrange("p (d o) -> p d o", o=OC),
                            in0=s_sb[:, :].rearrange("p (d o) -> p d o", o=OC),
                            in1=qq[:, :].unsqueeze(1).broadcast_to([B, OD, OC]),
                            op=ALU.mult)
                        nc.vector.tensor_add(Vaccb[:, :], Vaccb[:, :], vt[:, :])
                    for g in range(4):
                        nc.sync.dma_start(out=V4[32 * g:32 * g + 32, :], in_=Vaccb[:, :])

            # ======== pass 1: s0 = sum_i u_hat / 64 ========
            # dense contraction over (i, k), split in half: the AllReduce of
            # the first half overlaps the matmuls of the second half.
            se_sb = cp.tile([B, OD2], bf16, tag="seh", name="se_sb")
            so_sb = cp.tile([B, OD2], bf16, tag="soh", name="so_sb")
            for half in range(2):
                sacc = pp.tile([B, OD2], f32, tag="sacc", name=f"sacc1_{half}")
                t0, t1_ = half * NJ // 2, (half + 1) * NJ // 2
                for tau in range(t0, t1_):
                    for ch in range(4):
                        nc.tensor.matmul(
                            sacc[0:B, ch * 512:(ch + 1) * 512],
                            lhsT=x2t[:, tau * B:(tau + 1) * B],
                            rhs=wl[:, tau * OD2 + ch * 512: tau * OD2 + (ch + 1) * 512],
                            start=(tau == t0), stop=(tau == t1_ - 1),
                            tile_position=(0, 0))
                start_ar(NUM_ROUTES + half - 1 if half else 0,
                         sacc, se_sb if half == 0 else so_sb)
            # combine the two halves (in place into se_sb)
            nc.sync.dma_start(out=se_sb[:, :], in_=ar_out[0][:, :])
            nc.sync.dma_start(out=so_sb[:, :], in_=ar_out[NUM_ROUTES][:, :])
            nc.vector.tensor_add(se_sb[:, :], se_sb[:, :], so_sb[:, :])
            squash(0, se_sb)

            # ======== passes 2..3: fused agreement/softmax/s ========
            # software-pipelined: stage A for quad q runs in iteration q,
            # stage B for quad q in iteration q+1.
            for t in range(1, NUM_ROUTES):
                sacc = pp.tile([B, OD2], f32, tag="sacc", name=f"sacc{t}")
                state = {}
                for it in range(NQ + 1):
                    if it < NQ:
                        jj, g = divmod(it, 2)
                        # stage A: 4 consolidated matmuls (K=64: 4i x 16k
                        # block-diag x; M=128: 4i x 32b), one big evac, x V
                        uh = pp.tile([128, OD2], f32, tag="acc", name=f"uh{t}_{it}")
                        for ch in range(4):
                            nc.tensor.matmul(
                                uh[:, ch * 512:(ch + 1) * 512],
                                lhsT=xg[64 * g:64 * g + 64, jj * 128:(jj + 1) * 128],
                                rhs=wl[64 * g:64 * g + 64,
                                       jj * OD2 + ch * 512: jj * OD2 + (ch + 1) * 512],
                                start=True, stop=True,
                                tile_position=(64 * g, 0),
                            )
                        uhsb = up.tile([128, OD2], bf16, tag="uhb", name=f"uhsb{t}_{it}")
                        nc.scalar.copy(uhsb[:, :], uh[:, :])
                        tmp = wp.tile([128, OD2], bf16, tag="tmp")
                        nc.vector.tensor_mul(tmp[:, :], uhsb[:, :], V4[:, :])
                        h1 = wp.tile([128, OD2 // 2], bf16, tag="h1")
                        nc.gpsimd.tensor_add(h1[:, :], tmp[:, :OD2 // 2], tmp[:, OD2 // 2:])
                        state[it] = (uhsb, h1)
                    if it >= 1:
                        q = it - 1
                        uhsb, h1 = state.pop(q)
                        # stage B: finish agreement, softmax, c-weight, s-accum
                        h2 = wp.tile([128, OD2 // 4], bf16, tag="h2")
                        nc.vector.tensor_add(h2[:, :], h1[:, :OD2 // 4], h1[:, OD2 // 4:])
                        agr = sp.tile([128, OC], f32, tag="agr")
                        nc.vector.tensor_reduce(
                            agr[:, :], h2[:, :].rearrange("p (d o) -> p o d", o=OC),
                            axis=mybir.AxisListType.X, op=ALU.add)
                        eB = sp.tile([128, OC], bf16, tag="eB")
                        Zs = sp.tile([128, 1], f32, tag="Zs")
                        # ACT's accum_out yields Z = sum_o exp(agr) for free
                        nc.scalar.activation(eB[:, :], agr[:, :], ACTF.Exp,
                                             accum_out=Zs[:, :])
                        rZ = sp.tile([128, 1], f32, tag="rZ")
                        nc.vector.reciprocal(rZ[:, :], Zs[:, :])
                        # fold 1/Z on the scalar engine: cB = eB * rZ
                        cB = sp.tile([128, OC], bf16, tag="cB")
                        nc.scalar.activation(cB[:, :], eB[:, :], ACTF.Copy,
                                             scale=rZ[:, :])
                        tmp2 = wp.tile([128, OD2], bf16, tag="tmp2")
                        nc.vector.tensor_tensor(
                            out=tmp2[:, :].rearrange("p (d o) -> p d o", o=OC),
                            in0=uhsb[:, :].rearrange("p (d o) -> p d o", o=OC),
                            in1=cB[:, :].unsqueeze(1).broadcast_to([128, OD, OC]),
                            op=ALU.mult)
                        for ch in range(4):
                            nc.tensor.matmul(
                                sacc[0:B, ch * 512:(ch + 1) * 512], lhsT=sel1[:, :],
                                rhs=tmp2[:, ch * 512:(ch + 1) * 512],
                                start=(q == 0), stop=(q == NQ - 1),
                                tile_position=(0, 0))
                s_sb = cp.tile([B, OD2], bf16, tag="seh", name=f"s_sb{t}")
                start_ar(t, sacc, s_sb)
                nc.sync.dma_start(out=s_sb[:, :], in_=ar_out[t][:, :])
                squash(t, s_sb)

    nc.compile()
    return nc


def _host_inputs(x, W):
    """Build per-core input maps (host-side relayout, not device time)."""
    W0 = np.asarray(W)[0]                       # [IC, OC, OD, KD]
    x = np.asarray(x)                           # [B, IC, KD]
    in_maps = []
    sel1 = np.zeros((128, 32), np.float32)
    for p in range(128):
        sel1[p, p % 32] = 1.0
    for c in range(NCORES):
        # W layout: row (i8, k) with i8 = g*4 + ii, col (jj, d, o).
        Wc = W0[c * ICC:(c + 1) * ICC].reshape(NJ, 8, OC, OD, KD)   # [jj, i8, o, d, k]
        WL = np.ascontiguousarray(Wc.transpose(1, 4, 0, 3, 2)       # [i8, k, jj, d, o]
                                  ).reshape(128, NJ * OD2)
        xc = x[:, c * ICC:(c + 1) * ICC, :].reshape(B, NJ, 2, 4, KD)  # [b, jj, g, ii, k]
        xt = xc.transpose(2, 3, 4, 1, 0)                              # [g, ii, k, jj, b]
        Xg = np.zeros((2, 4, KD, NJ, 4, B), np.float32)  # [g, ii_r, k, jj, ii_c, b]
        for ii in range(4):
            Xg[:, ii, :, :, ii, :] = xt[:, ii]
        XG = Xg.reshape(128, NJ * 128)
        xc8 = x[:, c * ICC:(c + 1) * ICC, :].reshape(B, NJ, 8, KD)    # [b, jj, i8, k]
        X2 = (np.ascontiguousarray(xc8.transpose(2, 3, 1, 0))         # [i8, k, jj, b]
              .reshape(128, NJ * B) / float(OC))
        in_maps.append({
            "WL": WL.astype(ml_dtypes.bfloat16),
            "XG": XG.astype(ml_dtypes.bfloat16),
            "X2": X2.astype(ml_dtypes.bfloat16),
            "SEL1": sel1.astype(ml_dtypes.bfloat16),
        })
    return in_maps


def kernel(x, W, _want_trace=False):
    from concourse.bass_utils import run_bass_kernel_spmd

    if "nc" not in _CACHE:
        _CACHE["nc"] = _build_program()
    nc = _CACHE["nc"]
    in_maps = _host_inputs(x, W)
    res = run_bass_kernel_spmd(nc, in_maps, core_ids=list(range(NCORES)),
                               trace=_want_trace)
    _CACHE["last_result"] = res
    out = np.asarray(res.results[0]["v_out"], np.float32)
    return out.reshape(B, OC, OD)


# revision 10
# speedup vs baseline: 1.1076x; 1.1076x over previous
"""CapsuleLayer dynamic-routing kernel for 8 Trainium2 NeuronCores.

Problem: x[32, 2048, 16], W[1, 2048, 64, 32, 16] -> v[32, 64, 32]
  u_hat = einsum('iodk,bik->biod', W[0], x)
  3 routing iterations (softmax over out_caps, squash over out_dim).

Sharding: in_caps (i) split 8 ways (256/core).  W shard is SBUF-resident in
bf16 with (d, o) column order so every DVE broadcast has innermost stride 1
(2x_1p mode).  Per routing pass the tensor engine recomputes u_hat with
consolidated K=64/M=128 matmuls (block-diagonal x in lhsT packs 4 in_caps per
matmul); the scalar engine evacuates PSUM to bf16 and folds the softmax
normalization (cB = exp(agr) * rZ); the vector engine runs all-bf16
tensor_tensor ops at 2x plus a halving-tree d-reduction (first halving on
gpsimd).  The quad loop is software-pipelined: stage A (u_hat matmuls, evac,
x V multiply) runs one quad ahead of stage B (reduce, softmax, c-weight,
selector matmul) so no engine queue blocks in-order on a cross-engine dep.
s_j is AllReduced across the 8 cores in bf16 (pass 1 splits the reduction in
half to overlap the first AR with the second half of the matmuls); squash +
softmax bookkeeping is replicated on every core.

Routing state trick: b_ij(t) = sum_d u_hat * (v_0+...+v_{t-1}), so no b_ij
state is carried - only the accumulated V (32x2048 bf16).
"""

import numpy as np
import ml_dtypes

B, IC, KD, OC, OD = 32, 2048, 16, 64, 32     # batch, in_caps, in_dim, out_caps, out_dim
NCORES = 8
ICC = IC // NCORES                            # 256 in_caps per core
NJ = ICC // 8                                 # 32 j-blocks (8 i per block)
OD2 = OC * OD                                 # 2048 flattened (d, o) columns
NUM_ROUTES = 3
NQ = 2 * NJ                                   # 64 quads (4 i each) per pass

_CACHE = {}


def _build_program():
    import concourse.bacc as bacc
    import concourse.tile as tile
    import concourse.mybir as mybir

    f32 = mybir.dt.float32
    bf16 = mybir.dt.bfloat16
    ALU = mybir.AluOpType
    ACTF = mybir.ActivationFunctionType

    nc = bacc.Bacc("TRN2", target_bir_lowering=False, debug=False, num_devices=NCORES)

    WL_d = nc.dram_tensor("WL", [128, NJ * OD2], bf16, kind="ExternalInput").ap()
    XG_d = nc.dram_tensor("XG", [128, NJ * 128], bf16, kind="ExternalInput").ap()
    X2_d = nc.dram_tensor("X2", [128, NJ * B], bf16, kind="ExternalInput").ap()
    SEL1_d = nc.dram_tensor("SEL1", [128, 32], bf16, kind="ExternalInput").ap()
    vout_d = nc.dram_tensor("v_out", [B, OD2], f32, kind="ExternalOutput").ap()

    with tile.TileContext(nc) as tc:
        with (
            tc.tile_pool(name="const", bufs=1) as cp,
            tc.tile_pool(name="uh", bufs=4) as up,
            tc.tile_pool(name="work", bufs=2) as wp,
            tc.tile_pool(name="small", bufs=2) as sp,
            tc.tile_pool(name="psum", bufs=1, space="PSUM") as pp,
            tc.tile_pool(name="dram", bufs=1, space="DRAM") as dp,
        ):
            # ---- resident inputs ----
            wl = cp.tile([128, NJ * OD2], bf16, tag="wl")
            for blk in range(8):
                w = NJ * OD2 // 8
                nc.sync.dma_start(out=wl[:, blk * w:(blk + 1) * w],
                                  in_=WL_d[:, blk * w:(blk + 1) * w])
            xg = cp.tile([128, NJ * 128], bf16, tag="xg")
            nc.sync.dma_start(out=xg[:, :], in_=XG_d[:, :])
            x2t = cp.tile([128, NJ * B], bf16, tag="x2t")
            nc.sync.dma_start(out=x2t[:, :], in_=X2_d[:, :])
            sel1 = cp.tile([128, 32], bf16, tag="sel1")
            nc.sync.dma_start(out=sel1[:, :], in_=SEL1_d[:, :])

            # ---- persistent state ----
            V4 = cp.tile([128, OD2], bf16, tag="V4")     # V replicated x4 part-groups
            Vaccb = cp.tile([B, OD2], bf16, tag="Vacc")  # running sum of v_t

            NAR = NUM_ROUTES + 1                          # pass-1 uses 2 half ARs
            ar_in = [dp.tile([B, OD2], bf16, tag=f"ari{t}", name=f"ari{t}") for t in range(NAR)]
            ar_out = [dp.tile([B, OD2], bf16, tag=f"aro{t}", name=f"aro{t}") for t in range(NAR)]

            def start_ar(t, src_psum, dst_sb):
                """Evacuate s-partial (psum, f32) -> bf16 sbuf -> allreduce."""
                nc.scalar.copy(dst_sb[:, :], src_psum[0:B, :])
                nc.sync.dma_start(out=ar_in[t][:, :], in_=dst_sb[:, :])
                nc.gpsimd.collective_compute(
                    "AllReduce", ALU.add,
                    replica_groups=[list(range(NCORES))],
                    ins=[ar_in[t].opt()],
                    outs=[ar_out[t].opt()],
                )

            def squash(t, s_sb):
                """v_t = squash(s_sb).  t<2: Vaccb += v_t, V4 <- replicate(Vaccb).
                t==2: DMA v_t to output in (o, d) order."""
                n2 = sp.tile([B, OC], f32, tag="n2", bufs=1)
                if t < NUM_ROUTES - 1:
                    # bf16 halving tree for ||s||^2 (fast; feeds agreements only)
                    sq = wp.tile([B, OD2], bf16, tag="tmp", name=f"sq{t}")
                    nc.vector.tensor_mul(sq[:, :], s_sb[:, :], s_sb[:, :])
                    sh1 = sp.tile([B, OD2 // 2], bf16, tag="sh1", bufs=1)
                    nc.vector.tensor_add(sh1[:, :], sq[:, :OD2 // 2], sq[:, OD2 // 2:])
                    sh2 = sp.tile([B, OD2 // 4], bf16, tag="sh2", bufs=1)
                    nc.vector.tensor_add(sh2[:, :], sh1[:, :OD2 // 4], sh1[:, OD2 // 4:])
                    nc.vector.tensor_reduce(
                        n2[:, :], sh2[:, :].rearrange("p (d o) -> p o d", o=OC),
                        axis=mybir.AxisListType.X, op=ALU.add)
                else:
                    # exact f32 path for the final output
                    sqf = wp.tile([B, OD2], f32, tag="fin32", name=f"sqf{t}", bufs=1)
                    nc.scalar.activation(sqf[:, :], s_sb[:, :], ACTF.Square)
                    nc.vector.tensor_reduce(
                        n2[:, :], sqf[:, :].rearrange("p (d o) -> p o d", o=OC),
                        axis=mybir.AxisListType.X, op=ALU.add)
                r0 = sp.tile([B, OC], f32, tag="r0", bufs=1)
                nc.scalar.activation(r0[:, :], n2[:, :], ACTF.Sqrt)
                # Newton polish: n = 0.5 * (r0 + n2 / r0)
                t1 = sp.tile([B, OC], f32, tag="t1", bufs=1)
                nc.vector.reciprocal(t1[:, :], r0[:, :])
                nc.vector.tensor_mul(t1[:, :], t1[:, :], n2[:, :])
                t2 = sp.tile([B, OC], f32, tag="t2", bufs=1)
                nc.vector.tensor_add(t2[:, :], t1[:, :], r0[:, :])
                nn = sp.tile([B, OC], f32, tag="nn", bufs=1)
                nc.vector.tensor_scalar_mul(nn[:, :], t2[:, :], 0.5)   # |s|
                den = sp.tile([B, OC], f32, tag="den", bufs=1)
                nc.vector.tensor_scalar_add(den[:, :], n2[:, :], 1.0)
                rec = sp.tile([B, OC], f32, tag="rec", bufs=1)
                nc.vector.reciprocal(rec[:, :], den[:, :])
                qq = sp.tile([B, OC], bf16, tag="qq", bufs=1)
                nc.vector.tensor_mul(qq[:, :], nn[:, :], rec[:, :])  # |s|/(1+|s|^2)
                if t == NUM_ROUTES - 1:
                    vtf = wp.tile([B, OD2], f32, tag="fin32", name="vtf", bufs=1)
                    nc.vector.tensor_tensor(
                        out=vtf[:, :].rearrange("p (o d) -> p o d", d=OD),
                        in0=s_sb[:, :].rearrange("p (d o) -> p o d", o=OC),
                        in1=qq[:, :].unsqueeze(2).broadcast_to([B, OC, OD]),
                        op=ALU.mult)
                    nc.sync.dma_start(out=vout_d[:, :], in_=vtf[:, :])
                else:
                    if t == 0:
                        nc.vector.tensor_tensor(
                            out=Vaccb[:, :].rearrange("p (d o) -> p d o", o=OC),
                            in0=s_sb[:, :].rearrange("p (d o) -> p d o", o=OC),
                            in1=qq[:, :].unsqueeze(1).broadcast_to([B, OD, OC]),
                            op=ALU.mult)
                    else:
                        vt = wp.tile([B, OD2], bf16, tag="tmp", name=f"vt{t}")
                        nc.vector.tensor_tensor(
                            out=vt[:, :].rearrange("p (d o) -> p d o", o=OC),
                            in0=s_sb[:, :].rearrange("p (d o) -> p d o", o=OC),
                            in1=qq[:, :].unsqueeze(1).broadcast_to([B, OD, OC]),
                            op=ALU.mult)
                        nc.vector.tensor_add(Vaccb[:, :], Vaccb[:, :], vt[:, :])
                    for g in range(4):
                        nc.sync.dma_start(out=V4[32 * g:32 * g + 32, :], in_=Vaccb[:, :])

            # ======== pass 1: s0 = sum_i u_hat / 64 ========
            # dense contraction over (i, k), split in half: the AllReduce of
            # the first half overlaps the matmuls of the second half.
            se_sb = cp.tile([B, OD2], bf16, tag="seh", name="se_sb")
            so_sb = cp.tile([B, OD2], bf16, tag="soh", name="so_sb")
            for half in range(2):
                sacc = pp.tile([B, OD2], f32, tag="sacc", name=f"sacc1_{half}")
                t0, t1_ = half * NJ // 2, (half + 1) * NJ // 2
                for tau in range(t0, t1_):
                    for ch in range(4):
                        nc.tensor.matmul(
                            sacc[0:B, ch * 512:(ch + 1) * 512],
                            lhsT=x2t[:, tau * B:(tau + 1) * B],
                            rhs=wl[:, tau * OD2 + ch * 512: tau * OD2 + (ch + 1) * 512],
                            start=(tau == t0), stop=(tau == t1_ - 1),
                            tile_position=(0, 0))
                start_ar(NUM_ROUTES + half - 1 if half else 0,
                         sacc, se_sb if half == 0 else so_sb)
            # combine the two halves (in place into se_sb)
            nc.sync.dma_start(out=se_sb[:, :], in_=ar_out[0][:, :])
            nc.sync.dma_start(out=so_sb[:, :], in_=ar_out[NUM_ROUTES][:, :])
            nc.vector.tensor_add(se_sb[:, :], se_sb[:, :], so_sb[:, :])
            squash(0, se_sb)

            # ======== passes 2..3: fused agreement/softmax/s ========
            # software-pipelined: stage A for quad q runs in iteration q,
            # stage B for quad q in iteration q+1.
            for t in range(1, NUM_ROUTES):
                sacc = pp.tile([B, OD2], f32, tag="sacc", name=f"sacc{t}")
                state = {}
                for it in range(NQ + 1):
                    if it < NQ:
                        jj, g = divmod(it, 2)
                        # stage A: 4 consolidated matmuls (K=64: 4i x 16k
                        # block-diag x; M=128: 4i x 32b), one big evac, x V
                        uh = pp.tile([128, OD2], f32, tag="acc", name=f"uh{t}_{it}")
                        for ch in range(4):
                            nc.tensor.matmul(
                                uh[:, ch * 512:(ch + 1) * 512],
                                lhsT=xg[64 * g:64 * g + 64, jj * 128:(jj + 1) * 128],
                                rhs=wl[64 * g:64 * g + 64,
                                       jj * OD2 + ch * 512: jj * OD2 + (ch + 1) * 512],
                                start=True, stop=True,
                                tile_position=(64 * g, 0),
                            )
                        uhsb = up.tile([128, OD2], bf16, tag="uhb", name=f"uhsb{t}_{it}")
                        nc.scalar.copy(uhsb[:, :], uh[:, :])
                        tmp = wp.tile([128, OD2], bf16, tag="tmp")
                        nc.vector.tensor_mul(tmp[:, :], uhsb[:, :], V4[:, :])
                        h1 = wp.tile([128, OD2 // 2], bf16, tag="h1")
                        nc.vector.tensor_add(h1[:, :], tmp[:, :OD2 // 2], tmp[:, OD2 // 2:])
                        state[it] = (uhsb, h1)
                    if it >= 1:
                        q = it - 1
                        uhsb, h1 = state.pop(q)
                        # stage B: finish agreement, softmax, c-weight, s-accum
                        h2 = wp.tile([128, OD2 // 4], bf16, tag="h2")
                        nc.vector.tensor_add(h2[:, :], h1[:, :OD2 // 4], h1[:, OD2 // 4:])
                        agr = sp.tile([128, OC], f32, tag="agr")
                        nc.vector.tensor_reduce(
                            agr[:, :], h2[:, :].rearrange("p (d o) -> p o d", o=OC),
                            axis=mybir.AxisListType.X, op=ALU.add)
                        eB = sp.tile([128, OC], bf16, tag="eB")
                        Zs = sp.tile([128, 1], f32, tag="Zs")
                        # ACT's accum_out yields Z = sum_o exp(agr) for free
                        nc.scalar.activation(eB[:, :], agr[:, :], ACTF.Exp,
                                             accum_out=Zs[:, :])
                        rZ = sp.tile([128, 1], f32, tag="rZ")
                        nc.vector.reciprocal(rZ[:, :], Zs[:, :])
                        # fold 1/Z on the scalar engine: cB = eB * rZ
                        cB = sp.tile([128, OC], bf16, tag="cB")
                        nc.scalar.activation(cB[:, :], eB[:, :], ACTF.Copy,
                                             scale=rZ[:, :])
                        tmp2 = wp.tile([128, OD2], bf16, tag="tmp2")
                        nc.vector.tensor_tensor(
                            out=tmp2[:, :].rearrange("p (d o) -> p d o", o=OC),
                            in0=uhsb[:, :].rearrange("p (d o) -> p d o", o=OC),
                            in1=cB[:, :].unsqueeze(1).broadcast_to([128, OD, OC]),
                            op=ALU.mult)
                        for ch in range(4):
                            nc.tensor.matmul(
                                sacc[0:B, ch * 512:(ch + 1) * 512], lhsT=sel1[:, :],
                                rhs=tmp2[:, ch * 512:(ch + 1) * 512],
                                start=(q == 0), stop=(q == NQ - 1),
                                tile_position=(0, 0))
                s_sb = cp.tile([B, OD2], bf16, tag="seh", name=f"s_sb{t}")
                start_ar(t, sacc, s_sb)
                nc.sync.dma_start(out=s_sb[:, :], in_=ar_out[t][:, :])
                squash(t, s_sb)

    nc.compile()
    return nc


def _host_inputs(x, W):
    """Build per-core input maps (host-side relayout, not device time)."""
    W0 = np.asarray(W)[0]                       # [IC, OC, OD, KD]
    x = np.asarray(x)                           # [B, IC, KD]
    in_maps = []
    sel1 = np.zeros((128, 32), np.float32)
    for p in range(128):
        sel1[p, p % 32] = 1.0
    for c in range(NCORES):
        # W layout: row (i8, k) with i8 = g*4 + ii, col (jj, d, o).
        Wc = W0[c * ICC:(c + 1) * ICC].reshape(NJ, 8, OC, OD, KD)   # [jj, i8, o, d, k]
        WL = np.ascontiguousarray(Wc.transpose(1, 4, 0, 3, 2)       # [i8, k, jj, d, o]
                                  ).reshape(128, NJ * OD2)
        xc = x[:, c * ICC:(c + 1) * ICC, :].reshape(B, NJ, 2, 4, KD)  # [b, jj, g, ii, k]
        xt = xc.transpose(2, 3, 4, 1, 0)                              # [g, ii, k, jj, b]
        Xg = np.zeros((2, 4, KD, NJ, 4, B), np.float32)  # [g, ii_r, k, jj, ii_c, b]
        for ii in range(4):
            Xg[:, ii, :, :, ii, :] = xt[:, ii]
        XG = Xg.reshape(128, NJ * 128)
        xc8 = x[:, c * ICC:(c + 1) * ICC, :].reshape(B, NJ, 8, KD)    # [b, jj, i8, k]
        X2 = (np.ascontiguousarray(xc8.transpose(2, 3, 1, 0))         # [i8, k, jj, b]
              .reshape(128, NJ * B) / float(OC))
        in_maps.append({
            "WL": WL.astype(ml_dtypes.bfloat16),
            "XG": XG.astype(ml_dtypes.bfloat16),
            "X2": X2.astype(ml_dtypes.bfloat16),
            "SEL1": sel1.astype(ml_dtypes.bfloat16),
        })
    return in_maps


def kernel(x, W, _want_trace=False):
    from concourse.bass_utils import run_bass_kernel_spmd

    if "nc" not in _CACHE:
        _CACHE["nc"] = _build_program()
    nc = _CACHE["nc"]
    in_maps = _host_inputs(x, W)
    res = run_bass_kernel_spmd(nc, in_maps, core_ids=list(range(NCORES)),
                               trace=_want_trace)
    _CACHE["last_result"] = res
    out = np.asarray(res.results[0]["v_out"], np.float32)
    return out.reshape(B, OC, OD)
